# revision 7
# baseline (speedup 1.0000x reference)
"""DeepseekV3 MoE gate kernel for 8 TRN2 NeuronCores.

Strategy: shard tokens 8192 -> 8 x 1024, replicate gate weight/bias.

GEMM (per core, logits^T[e,t] accumulated in PSUM at 2^16 scale):
  - x is host-transposed and host-split: x16 = fp16(x) (shipped fp16) and
    xl8 = e4m3((x - x16) * 2^13) (shipped fp8). No on-chip transposes.
  - pass 1 (fp16): w16s = fp16(w) * 2^16 (exact power-of-2 scale), moving
    x16 slices [128h, 512t], stationary w16s tiles [128h, 128e].
  - pass 2 (one fp8 DoubleRow matmul per (k, e-tile) = both correction
    terms at once over the full 512-token chunk):
    acc += w8^T xl8 + wl8^T x8, where w8 = e4m3(w16*8) (shipped),
    wl8 = e4m3((w - fp16(w)) * 2^16) (shipped), x8 = e4m3(x16) (shipped; ACT runs
    only sigmoids so the activation-function table never reloads). All products carry the same 2^16 scale as pass 1, so a single
    PSUM accumulator works and the 2^-16 descale folds into the sigmoid's
    scale argument. Numpy-verified rel err ~1e-6 (exact top-k on ref data).
  - epilogue: sigmoid(acc*2^-16) on ACT in [e,t], PE-transpose scores to
    [t,e] PSUM, then the whole grouped top-k chain runs on DVE only (no
    cross-engine ping-pong; semaphore hop latency killed the previous
    multi-engine chain on hardware), using fused scalar_tensor_tensor ops.
"""
import contextlib
import sys

for _p in ("/opt/trn_rl_repo", "/opt/pypackages"):
    if _p not in sys.path:
        sys.path.append(_p)

import numpy as np
import concourse.bacc as bacc
import concourse.mybir as mybir
import concourse.tile as tile
from concourse import bass_utils

F32 = mybir.dt.float32
F16 = mybir.dt.float16
F8 = mybir.dt.float8e4
DR = mybir.MatmulPerfMode.DoubleRow
AF = mybir.ActivationFunctionType
OP = mybir.AluOpType
AX = mybir.AxisListType

TOKENS, HIDDEN, E = 8192, 7168, 256
N_CORES = 8
T = TOKENS // N_CORES          # 1024 tokens per core
KT = HIDDEN // 128             # 56 k-tiles
CH = 2                         # chunks per core
CHT = T // CH                  # 512 tokens per chunk
GK = 8                         # k-tiles per streamed group
NG = KT // GK                  # 7 groups
SCALE = 2.0 ** 16              # pass-1/2 common scale
S_XL = 2.0 ** 13               # xl8 = e4m3(xl * S_XL); w8 = e4m3(w * 8)
ROUTED_SCALING = 2.5

_CACHE = {}


def _declare_io(nc):
    x16_d = nc.dram_tensor("x16", [128, CH, KT, CHT], F16, kind="ExternalInput").ap()
    xl8_d = nc.dram_tensor("xl8", [128, CH, KT, CHT], F8, kind="ExternalInput").ap()
    x8_d = nc.dram_tensor("x8", [128, CH, KT, CHT], F8, kind="ExternalInput").ap()
    w16_d = nc.dram_tensor("w16s", [128, KT, E], F16, kind="ExternalInput").ap()
    wl8_d = nc.dram_tensor("wl8", [128, KT, E], F8, kind="ExternalInput").ap()
    w8_d = nc.dram_tensor("w8", [128, KT, E], F8, kind="ExternalInput").ap()
    biasp_d = nc.dram_tensor("bias_plain", [E], F32, kind="ExternalInput").ap()
    ident_d = nc.dram_tensor("ident", [128, 128], F32, kind="ExternalInput").ap()
    out_d = nc.dram_tensor("out", [T, E], F16, kind="ExternalOutput").ap()
    return x16_d, xl8_d, x8_d, w16_d, wl8_d, w8_d, biasp_d, ident_d, out_d


def _make_pools(tc, ctx):
    return {
        "const": ctx.enter_context(tc.tile_pool(name="const", bufs=1)),
        "xf": ctx.enter_context(tc.tile_pool(name="xf", bufs=3)),
        "xq": ctx.enter_context(tc.tile_pool(name="xq", bufs=3)),
        "et": ctx.enter_context(tc.tile_pool(name="et", bufs=2)),
        "tk": ctx.enter_context(tc.tile_pool(name="tk", bufs=2)),
        "pacc": ctx.enter_context(tc.tile_pool(name="pacc", bufs=2, space="PSUM")),
        "pot": ctx.enter_context(tc.tile_pool(name="pot", bufs=4, space="PSUM")),
    }


def _body(nc, pools, x16_d, xl8_d, x8_d, w16_d, wl8_d, w8_d, biasp_d, ident_d, out_d):
    const, xf, xq = pools["const"], pools["xf"], pools["xq"]
    et, tkp = pools["et"], pools["tk"]
    pacc, pot = pools["pacc"], pools["pot"]

    w16s_t = const.tile([128, KT, E], F16, name="w16s_t")
    w8p_t = const.tile([128, 2, KT, E], F8, name="w8p_t")
    bias_rep = const.tile([128, E], F32, name="bias_rep")
    ident = const.tile([128, 128], F32, name="ident")
    nc.sync.dma_start(ident, ident_d)

    def stream_weights(g):
        ks = slice(g * GK, (g + 1) * GK)
        nc.sync.dma_start(w16s_t[:, ks], w16_d[:, ks])
        nc.sync.dma_start(w8p_t[:, 1, ks], wl8_d[:, ks])
        nc.sync.dma_start(w8p_t[:, 0, ks], w8_d[:, ks])

    for c in range(CH):
        accs = [pacc.tile([128, CHT], F32, name=f"acc{e}_{c}", tag=f"acc{e}")
                for e in (0, 1)]
        for g in range(NG):
            gsl = slice(g * GK, (g + 1) * GK)
            x16g = xf.tile([128, GK, CHT], F16, name=f"x16_{c}_{g}", tag="x16")
            src = x16_d[:, c, gsl, :]
            if c == 0 and g == 0:
                stream_weights(0)
                # halve the very first transfer so the first matmuls start
                # as soon as the leading k-tiles land
                nc.sync.dma_start(x16g[:, 0:GK // 2], src[:, 0:GK // 2])
                nc.sync.dma_start(x16g[:, GK // 2:], src[:, GK // 2:])
            else:
                nc.sync.dma_start(x16g, src)
            xqg = xq.tile([128, 2, GK, CHT], F8, name=f"xq_{c}_{g}", tag="xq")
            nc.sync.dma_start(xqg[:, 0], xl8_d[:, c, gsl, :])
            nc.sync.dma_start(xqg[:, 1], x8_d[:, c, gsl, :])
            if c == 0 and g < NG - 1:
                stream_weights(g + 1)
            if c == 0 and g == 0:
                nc.sync.dma_start(bias_rep,
                                  biasp_d[None, :].to_broadcast([128, E]))
            for ko in range(GK):
                k = g * GK + ko
                for e in (0, 1):
                    nc.tensor.matmul(accs[e], w16s_t[:, k, e * 128:(e + 1) * 128],
                                     x16g[:, ko], start=(k == 0), stop=False)
            for ko in range(GK):
                k = g * GK + ko
                last = (k == KT - 1)
                for e in (0, 1):
                    nc.tensor.matmul(
                        accs[e],
                        w8p_t[:, :, k, e * 128:(e + 1) * 128],
                        xqg[:, :, ko, :],
                        start=False, stop=last, perf_mode=DR)

        # epilogue: sigmoid on ACT, scores transposed to [t, e] PSUM, then
        # the whole top-k chain stays on DVE (fused ops, no engine hops)
        sc_et = et.tile([128, 2, CHT], F32, name=f"sc_{c}", tag="sc")
        for e in (0, 1):
            nc.scalar.activation(sc_et[:, e], accs[e], AF.Sigmoid, scale=1.0 / SCALE)
        for t4 in range(4):
            tsl = slice(t4 * 128, (t4 + 1) * 128)
            ot = pot.tile([128, 256], F32, name=f"ot_{c}_{t4}", tag="ot")
            for e in (0, 1):
                nc.tensor.transpose(ot[:, e * 128:(e + 1) * 128],
                                    sc_et[:, e, tsl], ident)
            # ot holds scores[t, e] in PSUM for the rest of the chain
            swb = tkp.tile([128, 256], F32, name=f"swb_{c}_{t4}", tag="swb")
            nc.vector.scalar_tensor_tensor(swb, ot, 0.0, bias_rep,
                                           op0=OP.add, op1=OP.add)
            swb_g = swb.rearrange("p (g s) -> p g s", s=32)

            m1 = tkp.tile([128, 8], F32, name=f"m1_{c}_{t4}", tag="m1")
            nc.vector.tensor_reduce(m1, swb_g, axis=AX.X, op=OP.max)
            swb2 = tkp.tile([128, 256], F32, name=f"swb2_{c}_{t4}", tag="swb2")
            nc.vector.match_replace(out=swb2, in_to_replace=m1,
                                    in_values=swb, imm_value=-1e30)
            gsum = tkp.tile([128, 8], F32, name=f"gsum_{c}_{t4}", tag="gsum")
            nc.vector.tensor_reduce(gsum,
                                    swb2.rearrange("p (g s) -> p g s", s=32),
                                    axis=AX.X, op=OP.max)
            nc.vector.tensor_tensor(gsum, gsum, m1, op=OP.add)
            g8 = tkp.tile([128, 8], F32, name=f"g8_{c}_{t4}", tag="g8")
            nc.vector.max(out=g8, in_=gsum)
            # swbm = (group kept ? swb : 0) in one fused op
            swbm = tkp.tile([128, 256], F32, name=f"swbm_{c}_{t4}", tag="swbm")
            nc.vector.scalar_tensor_tensor(
                swbm.rearrange("p (g s) -> p g s", s=32),
                gsum[:, :, None].to_broadcast([128, 8, 32]), g8[:, 3:4],
                swb_g, op0=OP.is_ge, op1=OP.mult)
            top8 = tkp.tile([128, 8], F32, name=f"top8_{c}_{t4}", tag="top8")
            nc.vector.max(out=top8, in_=swbm)
            # sel = (swbm >= top8[7]) * scores, with row-sum accumulator
            sel = tkp.tile([128, 256], F32, name=f"sel_{c}_{t4}", tag="sel")
            ssum = tkp.tile([128, 1], F32, name=f"ssum_{c}_{t4}", tag="ssum")
            nc.vector.scalar_tensor_tensor(sel, swbm, top8[:, 7:8], ot,
                                           op0=OP.is_ge, op1=OP.mult,
                                           accum_out=ssum)
            inv = tkp.tile([128, 1], F32, name=f"inv_{c}_{t4}", tag="inv")
            nc.vector.reciprocal(inv, ssum)
            ow = tkp.tile([128, 256], F16, name=f"ow_{c}_{t4}", tag="ow")
            nc.vector.tensor_scalar(ow, sel, inv, ROUTED_SCALING,
                                    op0=OP.mult, op1=OP.mult)
            r0 = c * CHT + t4 * 128
            nc.sync.dma_start(out_d[r0:r0 + 128, :], ow)


def _build():
    nc = bacc.Bacc("TRN2", target_bir_lowering=False, debug=False)
    aps = _declare_io(nc)
    with tile.TileContext(nc) as tc:
        with contextlib.ExitStack() as ctx:
            pools = _make_pools(tc, ctx)
            _body(nc, pools, *aps)
    nc.compile()
    return nc


def _lay_x(a, dtype):
    # [1024, 7168] -> [128hp, 2ch, 56k, 512t], contiguous
    return np.ascontiguousarray(
        a.reshape(CH, CHT, KT, 128).transpose(3, 0, 2, 1).astype(dtype, copy=False))


def _lay_w(a, dtype):
    # [256, 7168] -> [128hp, 56k, 256e], contiguous
    return np.ascontiguousarray(
        a.T.reshape(KT, 128, E).transpose(1, 0, 2).astype(dtype, copy=False))


def _make_in_maps(hidden_states, weight, e_score_correction_bias):
    f32 = np.float32
    f8 = mybir.dt.np(F8)
    x = np.asarray(hidden_states, f32)
    w = np.asarray(weight, f32)
    b = np.asarray(e_score_correction_bias, f32)

    w16 = w.astype(np.float16)
    w16s = (w16.astype(f32) * SCALE).astype(np.float16)
    wl8 = ((w - w16.astype(f32)) * SCALE).astype(f8)
    w8 = (w16.astype(f32) * 8.0).astype(f8)
    w16s_l = _lay_w(w16s, np.float16)
    wl8_l = _lay_w(wl8, f8)
    w8_l = _lay_w(w8, f8)
    ident_np = np.eye(128, dtype=f32)

    in_maps = []
    for i in range(N_CORES):
        xc = x[i * T:(i + 1) * T]
        x16 = xc.astype(np.float16)
        xl8 = ((xc - x16.astype(f32)) * S_XL).astype(f8)
        x8 = x16.astype(f8)
        in_maps.append({
            "x16": _lay_x(x16, np.float16),
            "xl8": _lay_x(xl8, f8),
            "x8": _lay_x(x8, f8),
            "w16s": w16s_l, "wl8": wl8_l, "w8": w8_l,
            "bias_plain": b, "ident": ident_np,
        })
    return in_maps


def kernel(hidden_states, weight, e_score_correction_bias):
    in_maps = _make_in_maps(hidden_states, weight, e_score_correction_bias)
    if "nc" not in _CACHE:
        _CACHE["nc"] = _build()
    nc = _CACHE["nc"]
    res = bass_utils.run_bass_kernel_spmd(nc, in_maps, core_ids=list(range(N_CORES)))
    return np.concatenate(
        [res.results[i]["out"].astype(np.float32) for i in range(N_CORES)], axis=0)


if __name__ == "__main__":
    rng = np.random.default_rng(0)
    hs = rng.standard_normal((TOKENS, HIDDEN)).astype(np.float32)
    w = (rng.standard_normal((E, HIDDEN)) * 0.02).astype(np.float32)
    b = (rng.standard_normal(E) * 0.1).astype(np.float32)
    out = kernel(hs, w, b)
    print(out.shape, out.dtype, np.isfinite(out).all())
